# revision 14
# baseline (speedup 1.0000x reference)
"""Trainium2 Bass kernel for nn_ApplyAttentionPolicyMap.

Reference computes out = concat(logits, pp_logits) @ fc1 where fc1 is a
4288x1858 one-hot column-selection map: out[b, j] = flat[b, sel[j]].

Strategy (8 NeuronCores, sharded over output columns by source row):
  * Host: lay the activations feature-major (xT [4288, 8192]) in bf16 so the
    selection becomes a row gather at half the HBM traffic (the policy map
    only moves data, so bf16 rounding bounds the relative error at 2^-9).
    Sort the 1858 output columns by their source row sel[j] and split them
    into 8 equal groups; core k receives the contiguous band of xT rows
    covering its group (about 1/8th of the input) plus the group's local
    row indices.  Fat multi-KB gather rows keep the SWDGE descriptor count
    tiny (the ~1.4us per-indirect-DMA cadence is what limited a
    batch-sharded variant, which needed 15 instructions on the path).
  * Device: one small index load, then two big indirect row-gathers
    HBM->SBUF (128 + 105 rows of 16KB), each chased by a direct whole-chunk
    HWDGE store of the gathered rows to the feature-major output shard on
    its own ring; the small chunk goes first since the first gather is a
    serial prefix that no store can overlap.  Coarse
    chunks win measurably over finer pipelining here: each extra indirect
    DMA costs ~1.4us of SWDGE issue cadence plus a per-chunk completion
    stall, and sub-~100-partition or partition-offset transfers collapse
    the SDMA descriptor spray onto a single engine (measured 25GB/s).
  * Host again: un-permute columns, restore batch-major layout and f32.
"""

import numpy as np
import ml_dtypes

import concourse.bacc as bacc
import concourse.bass as bass
import concourse.mybir as mybir
from concourse.bass_utils import run_bass_kernel_spmd

N_CORES = 8
B = 8192
SECTIONS = (8192,)                # single batch section (sum = B)
NS = len(SECTIONS)
IN_DIM = 64 * 64 + 8 * 24         # 4288
OUT_DIM = 1858
NCOL = (OUT_DIM + N_CORES - 1) // N_CORES  # 233 columns per core (padded)
NR = (128, NCOL - 128)            # rows per gather chunk (128, 105)


_DT = mybir.dt.bfloat16

_cached = {}


def _build_nc(r_max: int):
    nc = bacc.Bacc("TRN2")

    xs = [
        nc.declare_dram_parameter(f"xs{s}", [r_max, w], _DT, isOutput=False)
        for s, w in enumerate(SECTIONS)
    ]
    idx_d = nc.declare_dram_parameter("idx", [128, 2], mybir.dt.int32, isOutput=False)
    outs_d = [
        nc.declare_dram_parameter(f"out{s}", [NCOL, w], _DT, isOutput=True)
        for s, w in enumerate(SECTIONS)
    ]

    from contextlib import ExitStack

    # gather issue order: the small 105-row chunk first — it is the serial
    # prefix no store can overlap, so it should be the cheap one — then the
    # 128-row chunk, whose store drains at line rate while measurement ends
    ns = len(SECTIONS)
    order = [(s, 1) for s in range(ns)] + [(s, 0) for s in reversed(range(ns))]

    with (
        nc.sbuf_tensor("gath", [128, 2, B], _DT) as gath,
        nc.sbuf_tensor("idx_sb", [128, 2], mybir.dt.int32) as idx_sb,
        nc.semaphore("io") as io_sem,
        nc.semaphore("outs") as out_sem,
        nc.semaphore("outs2") as out2_sem,
        ExitStack() as stack,
        nc.Block() as block,
    ):
        gsem = {
            sc: stack.enter_context(nc.semaphore(f"g{sc[0]}{sc[1]}"))  # noqa: ANT232
            for sc in order
        }
        sec0 = [0, *np.cumsum(SECTIONS)[:-1].tolist()]  # section col starts

        def gslice(s, c, nr):
            return gath[0:nr, c, sec0[s] : sec0[s] + SECTIONS[s]]

        # whole-chunk stores only: sub-128-partition splits collapse the
        # HWDGE descriptor spray onto one SDMA engine (~25GB/s, measured).
        # Greedily balance store bytes across the two HWDGE rings; the
        # last-issued chunk goes to Sync so its final drain is smallest.
        size = {(s, c): NR[c] * SECTIONS[s] for s, c in order}
        sync_stores, scalar_stores = [order[-1]], []
        loads = [size[order[-1]], 0]
        for sc in sorted(order[:-1], key=lambda sc: -size[sc]):
            i = 0 if loads[0] <= loads[1] else 1
            (sync_stores, scalar_stores)[i].append(sc)
            loads[i] += size[sc]
        sync_stores.sort(key=order.index)
        scalar_stores.sort(key=order.index)

        @block.sync
        def _(s):
            # index load; everything hangs off this ~2.5us round trip.
            s.dma_start(idx_sb[:, :], idx_d[:, :]).then_inc(io_sem, 16)
            for sec, c in sync_stores:
                nr = NR[c]
                s.wait_ge(gsem[(sec, c)], 16)
                s.dma_start(
                    out=outs_d[sec][c * 128 : c * 128 + nr, :],
                    in_=gslice(sec, c, nr),
                ).then_inc(out_sem, 16)

        @block.gpsimd
        def _(g):
            g.wait_ge(io_sem, 16)
            for sec, c in order:
                nr = NR[c]
                g.indirect_dma_start(
                    out=gslice(sec, c, nr),
                    out_offset=None,
                    in_=xs[sec][:, :],
                    in_offset=bass.IndirectOffsetOnAxis(
                        ap=idx_sb[0:nr, c : c + 1], axis=0
                    ),
                ).then_inc(gsem[(sec, c)], 16)

        @block.scalar
        def _(s):
            for sec, c in scalar_stores:
                nr = NR[c]
                s.wait_ge(gsem[(sec, c)], 16)
                s.dma_start(
                    out=outs_d[sec][c * 128 : c * 128 + nr, :],
                    in_=gslice(sec, c, nr),
                ).then_inc(out2_sem, 16)

    nc.compile()
    return nc


def _get_nc(r_max: int):
    if r_max not in _cached:
        _cached[r_max] = _build_nc(r_max)
    return _cached[r_max]


def _extract_sel(fc1: np.ndarray):
    """Return sel[j] with fc1 == one_hot(sel), or None if fc1 is not an
    exact one-hot column-selection map."""
    if fc1.shape != (IN_DIM, OUT_DIM):
        return None
    sel = np.argmax(fc1, axis=0)
    ok = (fc1[sel, np.arange(OUT_DIM)] == 1.0).all()
    if not ok:
        return None
    # each column must have exactly one nonzero
    nnz = np.count_nonzero(fc1, axis=0)
    if not (nnz == 1).all():
        return None
    return sel.astype(np.int64)


def _plan_shards(sel: np.ndarray):
    """Assign output columns to cores by sorted source row.

    Returns (groups, starts, r_max):
      groups[k]: the output-column ids owned by core k (sorted by sel)
      starts[k]: first xT row of core k's contiguous input band
      r_max:     uniform band height (rows) across cores
    """
    order = np.argsort(sel, kind="stable")
    base, rem = divmod(OUT_DIM, N_CORES)
    groups, lo = [], 0
    for k in range(N_CORES):
        n = base + (1 if k < rem else 0)
        groups.append(order[lo : lo + n])
        lo += n
    r_max = 1
    for g in groups:
        rows = sel[g]
        r_max = max(r_max, int(rows.max() - rows.min() + 1))
    starts = []
    for g in groups:
        r0 = int(sel[g].min())
        starts.append(min(r0, IN_DIM - r_max))
    return groups, starts, r_max


def _build_idx_tensor(local_rows: np.ndarray) -> np.ndarray:
    """int32 [128, 2]: idx[p, c] = local_rows[c*128 + p] (0 for padding)."""
    pad = np.zeros(2 * 128, dtype=np.int32)
    pad[: local_rows.shape[0]] = local_rows.astype(np.int32)
    return pad.reshape(2, 128).T.copy()


def kernel(logits: np.ndarray, pp_logits: np.ndarray, fc1: np.ndarray) -> np.ndarray:
    logits = np.asarray(logits, dtype=np.float32)
    pp_logits = np.asarray(pp_logits, dtype=np.float32)
    fc1 = np.asarray(fc1, dtype=np.float32)
    b = logits.shape[0]
    flat = np.concatenate(
        [logits.reshape(b, 64 * 64), pp_logits.reshape(b, 8 * 24)], axis=1
    )

    sel = _extract_sel(fc1)
    if sel is None or b != B:
        # Degenerate input (fc1 not an exact selection map, or unexpected
        # batch) — fall back to the dense reference computation.
        return flat @ fc1

    groups, starts, r_max = _plan_shards(sel)
    nc = _get_nc(r_max)
    xT = np.ascontiguousarray(flat.T.astype(ml_dtypes.bfloat16))  # [4288, 8192]
    sec0 = np.concatenate([[0], np.cumsum(SECTIONS)[:-1]])

    in_maps = []
    for k in range(N_CORES):
        r0 = starts[k]
        band = xT[r0 : r0 + r_max]
        m = {
            f"xs{s}": np.ascontiguousarray(band[:, sec0[s] : sec0[s] + SECTIONS[s]])
            for s in range(NS)
        }
        m["idx"] = _build_idx_tensor(sel[groups[k]] - r0)
        in_maps.append(m)

    res = run_bass_kernel_spmd(nc, in_maps, list(range(N_CORES)))

    outT = np.empty((OUT_DIM, B), dtype=np.float32)
    for k in range(N_CORES):
        n = groups[k].shape[0]
        for s in range(NS):
            outT[groups[k], sec0[s] : sec0[s] + SECTIONS[s]] = (
                res.results[k][f"out{s}"][:n].astype(np.float32)
            )
    return np.ascontiguousarray(outT.T)


# revision 15
# speedup vs baseline: 1.0738x; 1.0738x over previous
"""Trainium2 Bass kernel for nn_ApplyAttentionPolicyMap.

Reference computes out = concat(logits, pp_logits) @ fc1 where fc1 is a
4288x1858 one-hot column-selection map: out[b, j] = flat[b, sel[j]].

Strategy (8 NeuronCores, sharded over output columns by source row):
  * Host: lay the activations feature-major (xT [4288, 8192]) in bf16 so the
    selection becomes a row gather at half the HBM traffic (the policy map
    only moves data, so bf16 rounding bounds the relative error at 2^-9).
    Sort the 1858 output columns by their source row sel[j] and split them
    into 8 equal groups; core k receives the contiguous band of xT rows
    covering its group (about 1/8th of the input) plus the group's local
    row indices.  Fat multi-KB gather rows keep the SWDGE descriptor count
    tiny (the ~1.4us per-indirect-DMA cadence is what limited a
    batch-sharded variant, which needed 15 instructions on the path).
  * Device: one small index load, then two big indirect row-gathers
    HBM->SBUF (128 + 105 rows of 16KB), each chased by a direct whole-chunk
    HWDGE store of the gathered rows to the feature-major output shard on
    its own ring; the small chunk goes first since the first gather is a
    serial prefix that no store can overlap.  Coarse
    chunks win measurably over finer pipelining here: each extra indirect
    DMA costs ~1.4us of SWDGE issue cadence plus a per-chunk completion
    stall, and sub-~100-partition or partition-offset transfers collapse
    the SDMA descriptor spray onto a single engine (measured 25GB/s).
  * Host again: un-permute columns, restore batch-major layout and f32.
"""

import numpy as np
import ml_dtypes

import concourse.bacc as bacc
import concourse.bass as bass
import concourse.mybir as mybir
from concourse.bass_utils import run_bass_kernel_spmd

N_CORES = 8
B = 8192
SECTIONS = (8192,)                # single batch section (sum = B)
NS = len(SECTIONS)
IN_DIM = 64 * 64 + 8 * 24         # 4288
OUT_DIM = 1858
NCOL = (OUT_DIM + N_CORES - 1) // N_CORES  # 233 columns per core (padded)
NR = (128, NCOL - 128)            # rows per gather chunk (128, 105)


_DT = mybir.dt.bfloat16

_cached = {}


def _build_nc(r_max: int):
    nc = bacc.Bacc("TRN2")

    xs = [
        nc.declare_dram_parameter(f"xs{s}", [r_max, w], _DT, isOutput=False)
        for s, w in enumerate(SECTIONS)
    ]
    idx_d = nc.declare_dram_parameter("idx", [128, 2], mybir.dt.int32, isOutput=False)
    outs_d = [
        nc.declare_dram_parameter(f"out{s}", [NCOL, w], _DT, isOutput=True)
        for s, w in enumerate(SECTIONS)
    ]

    from contextlib import ExitStack

    # gather issue order: the small 105-row chunk first — it is the serial
    # prefix no store can overlap, so it should be the cheap one — then the
    # 128-row chunk, whose store drains at line rate while measurement ends
    ns = len(SECTIONS)
    order = [(s, 1) for s in range(ns)] + [(s, 0) for s in reversed(range(ns))]

    with (
        nc.sbuf_tensor("gath", [128, 2, B], _DT) as gath,
        nc.sbuf_tensor("idx_sb", [128, 2], mybir.dt.int32) as idx_sb,
        nc.semaphore("io") as io_sem,
        nc.semaphore("outs") as out_sem,
        nc.semaphore("outs2") as out2_sem,
        ExitStack() as stack,
        nc.Block() as block,
    ):
        gsem = {
            sc: stack.enter_context(nc.semaphore(f"g{sc[0]}{sc[1]}"))  # noqa: ANT232
            for sc in order
        }
        sec0 = [0, *np.cumsum(SECTIONS)[:-1].tolist()]  # section col starts

        def gslice(s, c, nr):
            return gath[0:nr, c, sec0[s] : sec0[s] + SECTIONS[s]]

        # whole-chunk stores only: sub-128-partition splits collapse the
        # HWDGE descriptor spray onto one SDMA engine (~25GB/s, measured).
        # Greedily balance store bytes across the two HWDGE rings; the
        # last-issued chunk goes to Sync so its final drain is smallest.
        size = {(s, c): NR[c] * SECTIONS[s] for s, c in order}
        sync_stores, scalar_stores = [order[-1]], []
        loads = [size[order[-1]], 0]
        for sc in sorted(order[:-1], key=lambda sc: -size[sc]):
            i = 0 if loads[0] <= loads[1] else 1
            (sync_stores, scalar_stores)[i].append(sc)
            loads[i] += size[sc]
        sync_stores.sort(key=order.index)
        scalar_stores.sort(key=order.index)

        @block.sync
        def _(s):
            # index load; everything hangs off this ~2.5us round trip.
            s.dma_start(idx_sb[:, :], idx_d[:, :]).then_inc(io_sem, 16)
            for sec, c in sync_stores:
                nr = NR[c]
                s.wait_ge(gsem[(sec, c)], 16)
                s.dma_start(
                    out=outs_d[sec][c * 128 : c * 128 + nr, :],
                    in_=gslice(sec, c, nr),
                ).then_inc(out_sem, 16)

        @block.gpsimd
        def _(g):
            g.wait_ge(io_sem, 16)
            for sec, c in order:
                # gather all 128 partitions even for the 105-row chunk (the
                # zero-padded indices fetch a few harmless extra rows): a
                # sub-128-partition transfer leaves the partially-loaded
                # SDMA engines idle and measures ~260 GB/s vs ~400 GB/s
                g.indirect_dma_start(
                    out=gslice(sec, c, 128),
                    out_offset=None,
                    in_=xs[sec][:, :],
                    in_offset=bass.IndirectOffsetOnAxis(
                        ap=idx_sb[0:128, c : c + 1], axis=0
                    ),
                ).then_inc(gsem[(sec, c)], 16)

        @block.scalar
        def _(s):
            for sec, c in scalar_stores:
                nr = NR[c]
                s.wait_ge(gsem[(sec, c)], 16)
                s.dma_start(
                    out=outs_d[sec][c * 128 : c * 128 + nr, :],
                    in_=gslice(sec, c, nr),
                ).then_inc(out2_sem, 16)

    nc.compile()
    return nc


def _get_nc(r_max: int):
    if r_max not in _cached:
        _cached[r_max] = _build_nc(r_max)
    return _cached[r_max]


def _extract_sel(fc1: np.ndarray):
    """Return sel[j] with fc1 == one_hot(sel), or None if fc1 is not an
    exact one-hot column-selection map."""
    if fc1.shape != (IN_DIM, OUT_DIM):
        return None
    sel = np.argmax(fc1, axis=0)
    ok = (fc1[sel, np.arange(OUT_DIM)] == 1.0).all()
    if not ok:
        return None
    # each column must have exactly one nonzero
    nnz = np.count_nonzero(fc1, axis=0)
    if not (nnz == 1).all():
        return None
    return sel.astype(np.int64)


def _plan_shards(sel: np.ndarray):
    """Assign output columns to cores by sorted source row.

    Returns (groups, starts, r_max):
      groups[k]: the output-column ids owned by core k (sorted by sel)
      starts[k]: first xT row of core k's contiguous input band
      r_max:     uniform band height (rows) across cores
    """
    order = np.argsort(sel, kind="stable")
    base, rem = divmod(OUT_DIM, N_CORES)
    groups, lo = [], 0
    for k in range(N_CORES):
        n = base + (1 if k < rem else 0)
        groups.append(order[lo : lo + n])
        lo += n
    r_max = 1
    for g in groups:
        rows = sel[g]
        r_max = max(r_max, int(rows.max() - rows.min() + 1))
    starts = []
    for g in groups:
        r0 = int(sel[g].min())
        starts.append(min(r0, IN_DIM - r_max))
    return groups, starts, r_max


def _build_idx_tensor(local_rows: np.ndarray) -> np.ndarray:
    """int32 [128, 2]: idx[p, c] = local_rows[c*128 + p] (0 for padding)."""
    pad = np.zeros(2 * 128, dtype=np.int32)
    pad[: local_rows.shape[0]] = local_rows.astype(np.int32)
    return pad.reshape(2, 128).T.copy()


def kernel(logits: np.ndarray, pp_logits: np.ndarray, fc1: np.ndarray) -> np.ndarray:
    logits = np.asarray(logits, dtype=np.float32)
    pp_logits = np.asarray(pp_logits, dtype=np.float32)
    fc1 = np.asarray(fc1, dtype=np.float32)
    b = logits.shape[0]
    flat = np.concatenate(
        [logits.reshape(b, 64 * 64), pp_logits.reshape(b, 8 * 24)], axis=1
    )

    sel = _extract_sel(fc1)
    if sel is None or b != B:
        # Degenerate input (fc1 not an exact selection map, or unexpected
        # batch) — fall back to the dense reference computation.
        return flat @ fc1

    groups, starts, r_max = _plan_shards(sel)
    nc = _get_nc(r_max)
    xT = np.ascontiguousarray(flat.T.astype(ml_dtypes.bfloat16))  # [4288, 8192]
    sec0 = np.concatenate([[0], np.cumsum(SECTIONS)[:-1]])

    in_maps = []
    for k in range(N_CORES):
        r0 = starts[k]
        band = xT[r0 : r0 + r_max]
        m = {
            f"xs{s}": np.ascontiguousarray(band[:, sec0[s] : sec0[s] + SECTIONS[s]])
            for s in range(NS)
        }
        m["idx"] = _build_idx_tensor(sel[groups[k]] - r0)
        in_maps.append(m)

    res = run_bass_kernel_spmd(nc, in_maps, list(range(N_CORES)))

    outT = np.empty((OUT_DIM, B), dtype=np.float32)
    for k in range(N_CORES):
        n = groups[k].shape[0]
        for s in range(NS):
            outT[groups[k], sec0[s] : sec0[s] + SECTIONS[s]] = (
                res.results[k][f"out{s}"][:n].astype(np.float32)
            )
    return np.ascontiguousarray(outT.T)
